# revision 32
# baseline (speedup 1.0000x reference)
"""Diversity7 loss kernel for Trainium2 (8 NeuronCores, Bass/Tile).

Math (per batch row b):
  p_m   = softmax(x_m / T)                          m = 0..6, C = 1000 classes
  v_m   = (p_m - mean(p_m)) / ||p_m - mean(p_m)||   (mean(p_m) = 1/C exactly)
  q_b   = || sum_m v_m ||^2
  loss  = SCALE * mean_b((q_b - M) / 2)

Device-side restructuring (all f32):
  e    = exp(x/T)                  (ACT pass, accum_out gives Se = sum e)
  dev2 = sum_c (e - Se/C)^2        (centered -> no catastrophic cancellation;
                                    split between ACT Square+accum and DVE
                                    affine_mul_reduce to balance engines)
  r2   = dev2/Se^2;  inv_r = rsqrt(r2) via magic-constant seed + 3 Newton
         steps (DVE-only; keeps ACT off the per-row-tile critical path)
  g    = inv_r/Se;   h = -inv_r/C
  s    = sum_m (g_m*e_m + h_m)      (fused affine_then_add chain on DVE;
                                     each g*e + h == centered normalized v_m)
  q    = sum_c s^2                  (fused affine_mul_reduce, DVE)
Emission is software-pipelined (row-tile rt+1's loads/exps issue before
row-tile rt's scalar math) so the FIFO engine queues never starve.
Host finishes in f64: loss = SCALE * mean((q-7)/2).

Sharding: data-parallel over batch. 8 cores x 512 rows; each core sees
[512,1000] slices of the 7 logit tensors and emits q for its rows as [128,4]
(partition p, row-tile rt) -> global row = core*512 + rt*128 + p.
`targets` is accepted and ignored (unused by the reference loss).
"""

import sys

import numpy as np

if "/opt/trn_rl_repo" not in sys.path:
    sys.path.insert(0, "/opt/trn_rl_repo")

import concourse.bass as bass
import concourse.tile as tile
from concourse import bacc, mybir
from concourse.bass_utils import run_bass_kernel_spmd


def _patch_act_tables() -> None:
    """Make Exp/Ln/Square resolve only via natural_log_exp_and_others so the
    kernel needs exactly one ACT table load (the default first-fit choice
    thrashes exp_and_others <-> natural_log sets, ~1.3us per switch)."""
    import concourse.hw_specs as hw_specs

    if getattr(hw_specs, "_diversity7_patched", False):
        return
    orig = hw_specs.get_activation_tables

    def patched(module_arch):
        tables = orig(module_arch)
        keep = "natural_log_exp_and_others"
        if keep in tables:
            only = {
                mybir.ActivationFunctionType.Exp,
                mybir.ActivationFunctionType.Ln,
                mybir.ActivationFunctionType.Square,
            }
            for name, funcs in tables.items():
                if name != keep:
                    funcs -= only
        return tables

    hw_specs.get_activation_tables = patched
    bacc.get_activation_tables = patched
    hw_specs._diversity7_patched = True

T = 20.0
SCALE = 0.3
C = 1000
M = 7
N_CORES = 8
ROWS_PER_CORE = 512
RT = ROWS_PER_CORE // 128  # row-tiles per core
MT = M * RT  # 28 (model, row-tile) pairs

# Engine balance tunables. GpSimd is kept OFF the big [128,1000] ops: its
# SBUF traffic contends with DVE's read ports and slows DVE ~2x (measured
# tensor_scalar 800ns -> 1475ns when pool runs big tensor_tensor underneath).
ACT_DEV2_MODELS = (4, 5, 6)  # last models -> DVE's dev2 work starts early

F32 = mybir.dt.float32
AF = mybir.ActivationFunctionType
ALU = mybir.AluOpType


def _is_act_dev2(m: int) -> bool:
    return m in ACT_DEV2_MODELS


def _build_program() -> bass.Bass:
    _patch_act_tables()
    nc = bacc.Bacc()
    xs = [
        nc.declare_dram_parameter(f"x{m}", [ROWS_PER_CORE, C], F32, isOutput=False)
        for m in range(M)
    ]
    # Per-column rescale for dev2: ACT columns hold +sum(e-eb)^2, DVE columns
    # hold -C*sum(e-eb)*e; colscale is 1.0 / -1/C respectively.
    colscale_in = nc.declare_dram_parameter("colscale", [128, MT], F32, isOutput=False)
    q_out = nc.declare_dram_parameter("q_out", [128, RT], F32, isOutput=True)

    with tile.TileContext(nc) as tc:
        with (
            tc.tile_pool(name="xp", bufs=6) as xp,
            tc.tile_pool(name="ep", bufs=1) as ep,
            tc.tile_pool(name="sp", bufs=2) as sp,
            tc.tile_pool(name="trp", bufs=3) as trp,
            tc.tile_pool(name="smp", bufs=1) as smp,
            tc.tile_pool(name="psp", bufs=2, space="PSUM") as psp,
            tc.tile_pool(name="qp", bufs=1) as qp,
        ):
            I32 = mybir.dt.int32
            q = qp.tile([128, RT], F32)
            colscale = smp.tile([128, MT], F32, tag="colscale")
            nc.sync.dma_start(colscale[:], colscale_in[:])
            # int consts for the magic-rsqrt seed
            one_i = smp.tile([128, 1], I32, tag="one_i")
            nc.vector.memset(one_i[:], 1)
            magic_t = smp.tile([128, M], I32, tag="magic_t")
            nc.vector.memset(magic_t[:], 0x5F3759DF)

            def phase1(rt: int):
                Se = smp.tile([128, M], F32, tag="Se", bufs=2, name=f"Se{rt}")
                dev2 = smp.tile([128, M], F32, tag="dev2", bufs=2, name=f"dev2{rt}")
                es: list[bass.AP] = []
                for m in range(M):
                    k = rt * M + m
                    x = xp.tile([128, C], F32, tag="x", name=f"x_{k}")
                    nc.sync.dma_start(x[:], xs[m][rt * 128 : (rt + 1) * 128, :])
                    e = ep.tile([128, C], F32, tag=f"e{m}", bufs=3, name=f"e_{k}")
                    nc.scalar.activation(
                        e[:], x[:], AF.Exp, bias=0.0, scale=1.0 / T,
                        accum_out=Se[:, m : m + 1],
                    )
                    trash = trp.tile([128, C], F32, tag="trash", name=f"tr_{k}")
                    if _is_act_dev2(m):
                        # dev2 = sum (e - Se/C)^2 on ACT. The rounded -1/C in
                        # negSeC only enters quadratically (sum(e-eb) == 0).
                        negSeC = smp.tile([128, 1], F32, tag=f"negSeC{k}",
                                          name=f"negSeC{k}")
                        nc.gpsimd.tensor_scalar_mul(
                            negSeC[:], Se[:, m : m + 1], -1.0 / C
                        )
                        nc.scalar.activation(
                            trash[:], e[:], AF.Square, bias=negSeC[:], scale=1.0,
                            accum_out=dev2[:, m : m + 1],
                        )
                    else:
                        # accum = sum (-C*e + Se)*e == -C*(Se2 - Se^2/C);
                        # scale/bias exact in f32, 1/C applied via colscale.
                        nc.vector.affine_mul_reduce(
                            out=trash[:], accum_out=dev2[:, m : m + 1],
                            in0=e[:], in1=e[:], scale=-float(C),
                            bias=Se[:, m : m + 1],
                        )
                    es.append(e)
                return Se, dev2, es

            def phase2_3(rt: int, Se, dev2, es: list[bass.AP]):
                sl = slice(rt * M, (rt + 1) * M)
                # Per-row scalars for this row-tile ([128, 7] slices).
                invSe = smp.tile([128, M], F32, tag="invSe", bufs=2, name=f"invSe{rt}")
                nc.vector.reciprocal(invSe[:], Se[:])
                t0 = smp.tile([128, M], F32, tag="t0", name=f"t0_{rt}")
                nc.vector.tensor_tensor(t0[:], dev2[:], invSe[:], ALU.mult)
                t1 = smp.tile([128, M], F32, tag="t1", name=f"t1_{rt}")
                nc.vector.tensor_tensor(t1[:], t0[:], invSe[:], ALU.mult)
                r2 = smp.tile([128, M], F32, tag="r2", name=f"r2_{rt}")
                nc.vector.tensor_tensor(r2[:], t1[:], colscale[:, sl], ALU.mult)
                # DVE-only rsqrt: magic-constant seed + 3 Newton steps
                # (keeps ACT out of the per-row-tile critical path).
                half_i = smp.tile([128, M], I32, tag="half_i", name=f"half_i{rt}")
                nc.vector.tensor_scalar(
                    half_i[:], r2[:].bitcast(I32), one_i[:, 0:1], None,
                    op0=ALU.logical_shift_right,
                )
                seed_i = smp.tile([128, M], I32, tag="seed_i", name=f"seed_i{rt}")
                nc.vector.tensor_tensor(
                    seed_i[:], magic_t[:], half_i[:], ALU.subtract
                )
                y = seed_i[:].bitcast(F32)
                for it in range(3):
                    ysq = smp.tile([128, M], F32, tag="ysq", bufs=2, name=f"ysq{rt}_{it}")
                    nc.vector.tensor_tensor(ysq[:], y, y, ALU.mult)
                    zy = smp.tile([128, M], F32, tag="zy", bufs=2, name=f"zy{rt}_{it}")
                    nc.vector.tensor_tensor(zy[:], r2[:], ysq[:], ALU.mult)
                    nrc = smp.tile([128, M], F32, tag="nrc", bufs=2, name=f"nrc{rt}_{it}")
                    nc.vector.tensor_scalar(
                        nrc[:], zy[:], -0.5, 1.5, op0=ALU.mult, op1=ALU.add
                    )
                    yn = smp.tile([128, M], F32, tag="invr", bufs=3, name=f"invr{rt}_{it}")
                    nc.vector.tensor_tensor(yn[:], y, nrc[:], ALU.mult)
                    y = yn[:]
                g = smp.tile([128, M], F32, tag="g", name=f"g{rt}")
                nc.vector.tensor_tensor(g[:], y, invSe[:], ALU.mult)
                h = smp.tile([128, M], F32, tag="h", name=f"h{rt}")
                nc.vector.tensor_scalar_mul(h[:], y, -1.0 / C)

                # s = sum_m (g_m*e_m + h_m) via fused affine_then_add chain;
                # each g*e + h == v_m (centered, normalized).
                s_prev = None
                for m in range(M):
                    s_new = sp.tile([128, C], F32, tag="s", bufs=3, name=f"s{rt}_{m}")
                    if m == 0:
                        nc.vector.tensor_scalar(
                            s_new[:], es[0][:], g[:, 0:1], h[:, 0:1],
                            op0=ALU.mult, op1=ALU.add,
                        )
                    else:
                        nc.vector.affine_then_add(
                            s_new[:], es[m][:], s_prev[:], g[:, m : m + 1],
                            h[:, m : m + 1],
                        )
                    s_prev = s_new
                trash2 = trp.tile([128, C], F32, tag="trash", name=f"tr2_{rt}")
                nc.vector.affine_mul_reduce(
                    out=trash2[:], accum_out=q[:, rt : rt + 1],
                    in0=s_prev[:], in1=s_prev[:], scale=1.0, bias=0.0,
                )

            # Software pipeline: emit row-tile rt+1's phase 1 BEFORE row-tile
            # rt's scalar math + chain, so the (FIFO) DVE queue always has
            # ready phase-1 work at row-tile boundaries.
            DEPTH = 2
            pending = []
            for rt in range(RT):
                pending.append((rt, *phase1(rt)))
                if len(pending) > DEPTH:
                    phase2_3(*pending.pop(0))
            for args in pending:
                phase2_3(*args)
            nc.sync.dma_start(q_out[:], q[:])
    return nc


_NC_CACHE: bass.Bass | None = None


def _get_program() -> bass.Bass:
    global _NC_CACHE
    if _NC_CACHE is None:
        nc = _build_program()
        nc.finalize()
        _NC_CACHE = nc
    return _NC_CACHE


def _colscale_np() -> np.ndarray:
    row = np.empty((MT,), dtype=np.float32)
    for rt in range(RT):
        for m in range(M):
            row[rt * M + m] = 1.0 if _is_act_dev2(m) else -1.0 / C
    return np.broadcast_to(row, (128, MT)).copy()


def run_device_part(inputs: dict[str, np.ndarray], **run_kwargs):
    """Run the bass kernel; returns (q_all [4096] f64 row-major, results)."""
    nc = _get_program()
    core_ids = list(range(N_CORES))
    colscale = _colscale_np()
    in_maps = []
    for c in range(N_CORES):
        lo, hi = c * ROWS_PER_CORE, (c + 1) * ROWS_PER_CORE
        im = {
            f"x{m}": np.ascontiguousarray(
                inputs[f"outputs{m + 1}"][lo:hi], dtype=np.float32
            )
            for m in range(M)
        }
        im["colscale"] = colscale
        in_maps.append(im)
    res = run_bass_kernel_spmd(nc, in_maps, core_ids, **run_kwargs)
    qs = []
    for c in range(N_CORES):
        qc = np.asarray(res.results[c]["q_out"])  # [128, RT]
        qs.append(qc.T.reshape(-1))  # row = rt*128 + p order
    q_all = np.concatenate(qs).astype(np.float64)  # row = c*512 + rt*128 + p
    return q_all, res


def kernel(**inputs: np.ndarray) -> np.ndarray:
    q_all, _ = run_device_part(inputs)
    loss = SCALE * np.mean((q_all - float(M)) / 2.0)
    return np.float32(loss)


# revision 33
# speedup vs baseline: 1.0286x; 1.0286x over previous
"""Diversity7 loss kernel for Trainium2 (8 NeuronCores, Bass/Tile).

Math (per batch row b):
  p_m   = softmax(x_m / T)                          m = 0..6, C = 1000 classes
  v_m   = (p_m - mean(p_m)) / ||p_m - mean(p_m)||   (mean(p_m) = 1/C exactly)
  q_b   = || sum_m v_m ||^2
  loss  = SCALE * mean_b((q_b - M) / 2)

Device-side restructuring (all f32):
  e    = exp(x/T)                  (ACT pass, accum_out gives Se = sum e)
  dev2 = sum_c (e - Se/C)^2        (centered -> no catastrophic cancellation;
                                    split between ACT Square+accum and DVE
                                    affine_mul_reduce to balance engines)
  r2   = dev2/Se^2;  inv_r = rsqrt(r2) via magic-constant seed + 3 Newton
         steps (DVE-only; keeps ACT off the per-row-tile critical path)
  g    = inv_r/Se;   h = -inv_r/C
  s    = sum_m (g_m*e_m + h_m)      (fused affine_then_add chain on DVE;
                                     each g*e + h == centered normalized v_m)
  q    = sum_c s^2                  (fused affine_mul_reduce, DVE)
Emission is software-pipelined (row-tile rt+1's loads/exps issue before
row-tile rt's scalar math) so the FIFO engine queues never starve.
Host finishes in f64: loss = SCALE * mean((q-7)/2).

Sharding: data-parallel over batch. 8 cores x 512 rows; each core sees
[512,1000] slices of the 7 logit tensors and emits q for its rows as [128,4]
(partition p, row-tile rt) -> global row = core*512 + rt*128 + p.
`targets` is accepted and ignored (unused by the reference loss).
"""

import sys

import numpy as np

if "/opt/trn_rl_repo" not in sys.path:
    sys.path.insert(0, "/opt/trn_rl_repo")

import concourse.bass as bass
import concourse.tile as tile
from concourse import bacc, mybir
from concourse.bass_utils import run_bass_kernel_spmd


def _patch_act_tables() -> None:
    """Make Exp/Ln/Square resolve only via natural_log_exp_and_others so the
    kernel needs exactly one ACT table load (the default first-fit choice
    thrashes exp_and_others <-> natural_log sets, ~1.3us per switch)."""
    import concourse.hw_specs as hw_specs

    if getattr(hw_specs, "_diversity7_patched", False):
        return
    orig = hw_specs.get_activation_tables

    def patched(module_arch):
        tables = orig(module_arch)
        keep = "natural_log_exp_and_others"
        if keep in tables:
            only = {
                mybir.ActivationFunctionType.Exp,
                mybir.ActivationFunctionType.Ln,
                mybir.ActivationFunctionType.Square,
            }
            for name, funcs in tables.items():
                if name != keep:
                    funcs -= only
        return tables

    hw_specs.get_activation_tables = patched
    bacc.get_activation_tables = patched
    hw_specs._diversity7_patched = True

T = 20.0
SCALE = 0.3
C = 1000
M = 7
N_CORES = 8
ROWS_PER_CORE = 512
RT = ROWS_PER_CORE // 128  # row-tiles per core
MT = M * RT  # 28 (model, row-tile) pairs

# Engine balance tunables. GpSimd is kept OFF the big [128,1000] ops: its
# SBUF traffic contends with DVE's read ports and slows DVE ~2x (measured
# tensor_scalar 800ns -> 1475ns when pool runs big tensor_tensor underneath).
ACT_DEV2_MODELS = (3, 4, 5, 6)  # last models -> DVE's dev2 work starts early

F32 = mybir.dt.float32
AF = mybir.ActivationFunctionType
ALU = mybir.AluOpType


def _is_act_dev2(m: int) -> bool:
    return m in ACT_DEV2_MODELS


def _build_program() -> bass.Bass:
    _patch_act_tables()
    nc = bacc.Bacc()
    xs = [
        nc.declare_dram_parameter(f"x{m}", [ROWS_PER_CORE, C], F32, isOutput=False)
        for m in range(M)
    ]
    # Per-column rescale for dev2: ACT columns hold +sum(e-eb)^2, DVE columns
    # hold -C*sum(e-eb)*e; colscale is 1.0 / -1/C respectively.
    colscale_in = nc.declare_dram_parameter("colscale", [128, MT], F32, isOutput=False)
    q_out = nc.declare_dram_parameter("q_out", [128, RT], F32, isOutput=True)

    with tile.TileContext(nc) as tc:
        with (
            tc.tile_pool(name="xp", bufs=6) as xp,
            tc.tile_pool(name="ep", bufs=1) as ep,
            tc.tile_pool(name="sp", bufs=2) as sp,
            tc.tile_pool(name="trp", bufs=3) as trp,
            tc.tile_pool(name="smp", bufs=1) as smp,
            tc.tile_pool(name="psp", bufs=2, space="PSUM") as psp,
            tc.tile_pool(name="qp", bufs=1) as qp,
        ):
            I32 = mybir.dt.int32
            q = qp.tile([128, RT], F32)
            colscale = smp.tile([128, MT], F32, tag="colscale")
            nc.sync.dma_start(colscale[:], colscale_in[:])
            # int consts for the magic-rsqrt seed
            one_i = smp.tile([128, 1], I32, tag="one_i")
            nc.vector.memset(one_i[:], 1)
            magic_t = smp.tile([128, M], I32, tag="magic_t")
            nc.vector.memset(magic_t[:], 0x5F3759DF)

            def phase1(rt: int):
                Se = smp.tile([128, M], F32, tag="Se", bufs=2, name=f"Se{rt}")
                dev2 = smp.tile([128, M], F32, tag="dev2", bufs=2, name=f"dev2{rt}")
                es: list[bass.AP] = []
                for m in range(M):
                    k = rt * M + m
                    x = xp.tile([128, C], F32, tag="x", name=f"x_{k}")
                    nc.sync.dma_start(x[:], xs[m][rt * 128 : (rt + 1) * 128, :])
                    e = ep.tile([128, C], F32, tag=f"e{m}", bufs=3, name=f"e_{k}")
                    nc.scalar.activation(
                        e[:], x[:], AF.Exp, bias=0.0, scale=1.0 / T,
                        accum_out=Se[:, m : m + 1],
                    )
                    trash = trp.tile([128, C], F32, tag="trash", name=f"tr_{k}")
                    if _is_act_dev2(m):
                        # dev2 = sum (e - Se/C)^2 on ACT. The rounded -1/C in
                        # negSeC only enters quadratically (sum(e-eb) == 0).
                        negSeC = smp.tile([128, 1], F32, tag=f"negSeC{k}",
                                          name=f"negSeC{k}")
                        nc.gpsimd.tensor_scalar_mul(
                            negSeC[:], Se[:, m : m + 1], -1.0 / C
                        )
                        nc.scalar.activation(
                            trash[:], e[:], AF.Square, bias=negSeC[:], scale=1.0,
                            accum_out=dev2[:, m : m + 1],
                        )
                    else:
                        # accum = sum (-C*e + Se)*e == -C*(Se2 - Se^2/C);
                        # scale/bias exact in f32, 1/C applied via colscale.
                        nc.vector.affine_mul_reduce(
                            out=trash[:], accum_out=dev2[:, m : m + 1],
                            in0=e[:], in1=e[:], scale=-float(C),
                            bias=Se[:, m : m + 1],
                        )
                    es.append(e)
                return Se, dev2, es

            def phase2_3(rt: int, Se, dev2, es: list[bass.AP]):
                sl = slice(rt * M, (rt + 1) * M)
                # Per-row scalars for this row-tile ([128, 7] slices).
                invSe = smp.tile([128, M], F32, tag="invSe", bufs=2, name=f"invSe{rt}")
                nc.vector.reciprocal(invSe[:], Se[:])
                t0 = smp.tile([128, M], F32, tag="t0", name=f"t0_{rt}")
                nc.vector.tensor_tensor(t0[:], dev2[:], invSe[:], ALU.mult)
                t1 = smp.tile([128, M], F32, tag="t1", name=f"t1_{rt}")
                nc.vector.tensor_tensor(t1[:], t0[:], invSe[:], ALU.mult)
                r2 = smp.tile([128, M], F32, tag="r2", name=f"r2_{rt}")
                nc.vector.tensor_tensor(r2[:], t1[:], colscale[:, sl], ALU.mult)
                # DVE-only rsqrt: magic-constant seed + 3 Newton steps
                # (keeps ACT out of the per-row-tile critical path).
                half_i = smp.tile([128, M], I32, tag="half_i", name=f"half_i{rt}")
                nc.vector.tensor_scalar(
                    half_i[:], r2[:].bitcast(I32), one_i[:, 0:1], None,
                    op0=ALU.logical_shift_right,
                )
                seed_i = smp.tile([128, M], I32, tag="seed_i", name=f"seed_i{rt}")
                nc.vector.tensor_tensor(
                    seed_i[:], magic_t[:], half_i[:], ALU.subtract
                )
                y = seed_i[:].bitcast(F32)
                for it in range(3):
                    ysq = smp.tile([128, M], F32, tag="ysq", bufs=2, name=f"ysq{rt}_{it}")
                    nc.vector.tensor_tensor(ysq[:], y, y, ALU.mult)
                    zy = smp.tile([128, M], F32, tag="zy", bufs=2, name=f"zy{rt}_{it}")
                    nc.vector.tensor_tensor(zy[:], r2[:], ysq[:], ALU.mult)
                    nrc = smp.tile([128, M], F32, tag="nrc", bufs=2, name=f"nrc{rt}_{it}")
                    nc.vector.tensor_scalar(
                        nrc[:], zy[:], -0.5, 1.5, op0=ALU.mult, op1=ALU.add
                    )
                    yn = smp.tile([128, M], F32, tag="invr", bufs=3, name=f"invr{rt}_{it}")
                    nc.vector.tensor_tensor(yn[:], y, nrc[:], ALU.mult)
                    y = yn[:]
                g = smp.tile([128, M], F32, tag="g", name=f"g{rt}")
                nc.vector.tensor_tensor(g[:], y, invSe[:], ALU.mult)
                h = smp.tile([128, M], F32, tag="h", name=f"h{rt}")
                nc.vector.tensor_scalar_mul(h[:], y, -1.0 / C)

                # s = sum_m (g_m*e_m + h_m) via fused affine_then_add chain;
                # each g*e + h == v_m (centered, normalized).
                s_prev = None
                for m in range(M):
                    s_new = sp.tile([128, C], F32, tag="s", bufs=3, name=f"s{rt}_{m}")
                    if m == 0:
                        nc.vector.tensor_scalar(
                            s_new[:], es[0][:], g[:, 0:1], h[:, 0:1],
                            op0=ALU.mult, op1=ALU.add,
                        )
                    else:
                        nc.vector.affine_then_add(
                            s_new[:], es[m][:], s_prev[:], g[:, m : m + 1],
                            h[:, m : m + 1],
                        )
                    s_prev = s_new
                trash2 = trp.tile([128, C], F32, tag="trash", name=f"tr2_{rt}")
                nc.vector.affine_mul_reduce(
                    out=trash2[:], accum_out=q[:, rt : rt + 1],
                    in0=s_prev[:], in1=s_prev[:], scale=1.0, bias=0.0,
                )

            # Software pipeline: emit row-tile rt+1's phase 1 BEFORE row-tile
            # rt's scalar math + chain, so the (FIFO) DVE queue always has
            # ready phase-1 work at row-tile boundaries.
            DEPTH = 1
            pending = []
            for rt in range(RT):
                pending.append((rt, *phase1(rt)))
                if len(pending) > DEPTH:
                    phase2_3(*pending.pop(0))
            for args in pending:
                phase2_3(*args)
            nc.sync.dma_start(q_out[:], q[:])
    return nc


_NC_CACHE: bass.Bass | None = None


def _get_program() -> bass.Bass:
    global _NC_CACHE
    if _NC_CACHE is None:
        nc = _build_program()
        nc.finalize()
        _NC_CACHE = nc
    return _NC_CACHE


def _colscale_np() -> np.ndarray:
    row = np.empty((MT,), dtype=np.float32)
    for rt in range(RT):
        for m in range(M):
            row[rt * M + m] = 1.0 if _is_act_dev2(m) else -1.0 / C
    return np.broadcast_to(row, (128, MT)).copy()


def run_device_part(inputs: dict[str, np.ndarray], **run_kwargs):
    """Run the bass kernel; returns (q_all [4096] f64 row-major, results)."""
    nc = _get_program()
    core_ids = list(range(N_CORES))
    colscale = _colscale_np()
    in_maps = []
    for c in range(N_CORES):
        lo, hi = c * ROWS_PER_CORE, (c + 1) * ROWS_PER_CORE
        im = {
            f"x{m}": np.ascontiguousarray(
                inputs[f"outputs{m + 1}"][lo:hi], dtype=np.float32
            )
            for m in range(M)
        }
        im["colscale"] = colscale
        in_maps.append(im)
    res = run_bass_kernel_spmd(nc, in_maps, core_ids, **run_kwargs)
    qs = []
    for c in range(N_CORES):
        qc = np.asarray(res.results[c]["q_out"])  # [128, RT]
        qs.append(qc.T.reshape(-1))  # row = rt*128 + p order
    q_all = np.concatenate(qs).astype(np.float64)  # row = c*512 + rt*128 + p
    return q_all, res


def kernel(**inputs: np.ndarray) -> np.ndarray:
    q_all, _ = run_device_part(inputs)
    loss = SCALE * np.mean((q_all - float(M)) / 2.0)
    return np.float32(loss)


# revision 34
# speedup vs baseline: 1.0365x; 1.0077x over previous
"""Diversity7 loss kernel for Trainium2 (8 NeuronCores, Bass/Tile).

Math (per batch row b):
  p_m   = softmax(x_m / T)                          m = 0..6, C = 1000 classes
  v_m   = (p_m - mean(p_m)) / ||p_m - mean(p_m)||   (mean(p_m) = 1/C exactly)
  q_b   = || sum_m v_m ||^2
  loss  = SCALE * mean_b((q_b - M) / 2)

Device-side restructuring (all f32):
  e    = exp(x/T)                  (ACT pass, accum_out gives Se = sum e)
  dev2 = sum_c (e - Se/C)^2        (centered -> no catastrophic cancellation;
                                    split between ACT Square+accum and DVE
                                    affine_mul_reduce to balance engines)
  r2   = dev2/Se^2;  inv_r = rsqrt(r2) via magic-constant seed + 3 Newton
         steps (DVE-only; keeps ACT off the per-row-tile critical path)
  g    = inv_r/Se;   h = -inv_r/C
  s    = sum_m (g_m*e_m + h_m)      (fused affine_then_add chain on DVE;
                                     each g*e + h == centered normalized v_m)
  q    = sum_c s^2                  (fused affine_mul_reduce, DVE)
Emission is software-pipelined (row-tile rt+1's loads/exps issue before
row-tile rt's scalar math) so the FIFO engine queues never starve.
Host finishes in f64: loss = SCALE * mean((q-7)/2).

Sharding: data-parallel over batch. 8 cores x 512 rows; each core sees
[512,1000] slices of the 7 logit tensors and emits q for its rows as [128,4]
(partition p, row-tile rt) -> global row = core*512 + rt*128 + p.
`targets` is accepted and ignored (unused by the reference loss).
"""

import sys

import numpy as np

if "/opt/trn_rl_repo" not in sys.path:
    sys.path.insert(0, "/opt/trn_rl_repo")

import concourse.bass as bass
import concourse.tile as tile
from concourse import bacc, mybir
from concourse.bass_utils import run_bass_kernel_spmd


def _patch_act_tables() -> None:
    """Make Exp/Ln/Square resolve only via natural_log_exp_and_others so the
    kernel needs exactly one ACT table load (the default first-fit choice
    thrashes exp_and_others <-> natural_log sets, ~1.3us per switch)."""
    import concourse.hw_specs as hw_specs

    if getattr(hw_specs, "_diversity7_patched", False):
        return
    orig = hw_specs.get_activation_tables

    def patched(module_arch):
        tables = orig(module_arch)
        keep = "natural_log_exp_and_others"
        if keep in tables:
            only = {
                mybir.ActivationFunctionType.Exp,
                mybir.ActivationFunctionType.Ln,
                mybir.ActivationFunctionType.Square,
            }
            for name, funcs in tables.items():
                if name != keep:
                    funcs -= only
        return tables

    hw_specs.get_activation_tables = patched
    bacc.get_activation_tables = patched
    hw_specs._diversity7_patched = True

T = 20.0
SCALE = 0.3
C = 1000
M = 7
N_CORES = 8
ROWS_PER_CORE = 512
RT = ROWS_PER_CORE // 128  # row-tiles per core
MT = M * RT  # 28 (model, row-tile) pairs

# Engine balance tunables. GpSimd is kept OFF the big [128,1000] ops: its
# SBUF traffic contends with DVE's read ports and slows DVE ~2x (measured
# tensor_scalar 800ns -> 1475ns when pool runs big tensor_tensor underneath).
ACT_DEV2_MODELS = (2, 3, 4, 5, 6)  # last models -> DVE's dev2 work starts early

F32 = mybir.dt.float32
AF = mybir.ActivationFunctionType
ALU = mybir.AluOpType


def _is_act_dev2(m: int) -> bool:
    return m in ACT_DEV2_MODELS


def _build_program() -> bass.Bass:
    _patch_act_tables()
    nc = bacc.Bacc()
    xs = [
        nc.declare_dram_parameter(f"x{m}", [ROWS_PER_CORE, C], F32, isOutput=False)
        for m in range(M)
    ]
    # Per-column rescale for dev2: ACT columns hold +sum(e-eb)^2, DVE columns
    # hold -C*sum(e-eb)*e; colscale is 1.0 / -1/C respectively.
    colscale_in = nc.declare_dram_parameter("colscale", [128, MT], F32, isOutput=False)
    q_out = nc.declare_dram_parameter("q_out", [128, RT], F32, isOutput=True)

    with tile.TileContext(nc) as tc:
        with (
            tc.tile_pool(name="xp", bufs=6) as xp,
            tc.tile_pool(name="ep", bufs=1) as ep,
            tc.tile_pool(name="sp", bufs=2) as sp,
            tc.tile_pool(name="trp", bufs=3) as trp,
            tc.tile_pool(name="smp", bufs=1) as smp,
            tc.tile_pool(name="psp", bufs=2, space="PSUM") as psp,
            tc.tile_pool(name="qp", bufs=1) as qp,
        ):
            I32 = mybir.dt.int32
            q = qp.tile([128, RT], F32)
            colscale = smp.tile([128, MT], F32, tag="colscale")
            nc.sync.dma_start(colscale[:], colscale_in[:])
            # int consts for the magic-rsqrt seed
            one_i = smp.tile([128, 1], I32, tag="one_i")
            nc.vector.memset(one_i[:], 1)
            magic_t = smp.tile([128, M], I32, tag="magic_t")
            nc.vector.memset(magic_t[:], 0x5F3759DF)

            def phase1(rt: int):
                Se = smp.tile([128, M], F32, tag="Se", bufs=2, name=f"Se{rt}")
                dev2 = smp.tile([128, M], F32, tag="dev2", bufs=2, name=f"dev2{rt}")
                es: list[bass.AP] = []
                for m in range(M):
                    k = rt * M + m
                    x = xp.tile([128, C], F32, tag="x", name=f"x_{k}")
                    nc.sync.dma_start(x[:], xs[m][rt * 128 : (rt + 1) * 128, :])
                    e = ep.tile([128, C], F32, tag=f"e{m}", bufs=3, name=f"e_{k}")
                    nc.scalar.activation(
                        e[:], x[:], AF.Exp, bias=0.0, scale=1.0 / T,
                        accum_out=Se[:, m : m + 1],
                    )
                    trash = trp.tile([128, C], F32, tag="trash", name=f"tr_{k}")
                    if _is_act_dev2(m):
                        # dev2 = sum (e - Se/C)^2 on ACT. The rounded -1/C in
                        # negSeC only enters quadratically (sum(e-eb) == 0).
                        negSeC = smp.tile([128, 1], F32, tag=f"negSeC{k}",
                                          name=f"negSeC{k}")
                        nc.gpsimd.tensor_scalar_mul(
                            negSeC[:], Se[:, m : m + 1], -1.0 / C
                        )
                        nc.scalar.activation(
                            trash[:], e[:], AF.Square, bias=negSeC[:], scale=1.0,
                            accum_out=dev2[:, m : m + 1],
                        )
                    else:
                        # accum = sum (-C*e + Se)*e == -C*(Se2 - Se^2/C);
                        # scale/bias exact in f32, 1/C applied via colscale.
                        nc.vector.affine_mul_reduce(
                            out=trash[:], accum_out=dev2[:, m : m + 1],
                            in0=e[:], in1=e[:], scale=-float(C),
                            bias=Se[:, m : m + 1],
                        )
                    es.append(e)
                return Se, dev2, es

            def phase2_3(rt: int, Se, dev2, es: list[bass.AP]):
                sl = slice(rt * M, (rt + 1) * M)
                # Per-row scalars for this row-tile ([128, 7] slices).
                invSe = smp.tile([128, M], F32, tag="invSe", bufs=2, name=f"invSe{rt}")
                nc.vector.reciprocal(invSe[:], Se[:])
                t0 = smp.tile([128, M], F32, tag="t0", name=f"t0_{rt}")
                nc.vector.tensor_tensor(t0[:], dev2[:], invSe[:], ALU.mult)
                t1 = smp.tile([128, M], F32, tag="t1", name=f"t1_{rt}")
                nc.vector.tensor_tensor(t1[:], t0[:], invSe[:], ALU.mult)
                r2 = smp.tile([128, M], F32, tag="r2", name=f"r2_{rt}")
                nc.vector.tensor_tensor(r2[:], t1[:], colscale[:, sl], ALU.mult)
                # DVE-only rsqrt: magic-constant seed + 3 Newton steps
                # (keeps ACT out of the per-row-tile critical path).
                half_i = smp.tile([128, M], I32, tag="half_i", name=f"half_i{rt}")
                nc.vector.tensor_scalar(
                    half_i[:], r2[:].bitcast(I32), one_i[:, 0:1], None,
                    op0=ALU.logical_shift_right,
                )
                seed_i = smp.tile([128, M], I32, tag="seed_i", name=f"seed_i{rt}")
                nc.vector.tensor_tensor(
                    seed_i[:], magic_t[:], half_i[:], ALU.subtract
                )
                y = seed_i[:].bitcast(F32)
                for it in range(3):
                    ysq = smp.tile([128, M], F32, tag="ysq", bufs=2, name=f"ysq{rt}_{it}")
                    nc.vector.tensor_tensor(ysq[:], y, y, ALU.mult)
                    zy = smp.tile([128, M], F32, tag="zy", bufs=2, name=f"zy{rt}_{it}")
                    nc.vector.tensor_tensor(zy[:], r2[:], ysq[:], ALU.mult)
                    nrc = smp.tile([128, M], F32, tag="nrc", bufs=2, name=f"nrc{rt}_{it}")
                    nc.vector.tensor_scalar(
                        nrc[:], zy[:], -0.5, 1.5, op0=ALU.mult, op1=ALU.add
                    )
                    yn = smp.tile([128, M], F32, tag="invr", bufs=3, name=f"invr{rt}_{it}")
                    nc.vector.tensor_tensor(yn[:], y, nrc[:], ALU.mult)
                    y = yn[:]
                g = smp.tile([128, M], F32, tag="g", name=f"g{rt}")
                nc.vector.tensor_tensor(g[:], y, invSe[:], ALU.mult)
                h = smp.tile([128, M], F32, tag="h", name=f"h{rt}")
                nc.vector.tensor_scalar_mul(h[:], y, -1.0 / C)

                # s = sum_m (g_m*e_m + h_m) via fused affine_then_add chain;
                # each g*e + h == v_m (centered, normalized).
                s_prev = None
                for m in range(M):
                    s_new = sp.tile([128, C], F32, tag="s", bufs=3, name=f"s{rt}_{m}")
                    if m == 0:
                        nc.vector.tensor_scalar(
                            s_new[:], es[0][:], g[:, 0:1], h[:, 0:1],
                            op0=ALU.mult, op1=ALU.add,
                        )
                    else:
                        nc.vector.affine_then_add(
                            s_new[:], es[m][:], s_prev[:], g[:, m : m + 1],
                            h[:, m : m + 1],
                        )
                    s_prev = s_new
                trash2 = trp.tile([128, C], F32, tag="trash", name=f"tr2_{rt}")
                nc.vector.affine_mul_reduce(
                    out=trash2[:], accum_out=q[:, rt : rt + 1],
                    in0=s_prev[:], in1=s_prev[:], scale=1.0, bias=0.0,
                )

            # Software pipeline: emit row-tile rt+1's phase 1 BEFORE row-tile
            # rt's scalar math + chain, so the (FIFO) DVE queue always has
            # ready phase-1 work at row-tile boundaries.
            DEPTH = 1
            pending = []
            for rt in range(RT):
                pending.append((rt, *phase1(rt)))
                if len(pending) > DEPTH:
                    phase2_3(*pending.pop(0))
            for args in pending:
                phase2_3(*args)
            nc.sync.dma_start(q_out[:], q[:])
    return nc


_NC_CACHE: bass.Bass | None = None


def _get_program() -> bass.Bass:
    global _NC_CACHE
    if _NC_CACHE is None:
        nc = _build_program()
        nc.finalize()
        _NC_CACHE = nc
    return _NC_CACHE


def _colscale_np() -> np.ndarray:
    row = np.empty((MT,), dtype=np.float32)
    for rt in range(RT):
        for m in range(M):
            row[rt * M + m] = 1.0 if _is_act_dev2(m) else -1.0 / C
    return np.broadcast_to(row, (128, MT)).copy()


def run_device_part(inputs: dict[str, np.ndarray], **run_kwargs):
    """Run the bass kernel; returns (q_all [4096] f64 row-major, results)."""
    nc = _get_program()
    core_ids = list(range(N_CORES))
    colscale = _colscale_np()
    in_maps = []
    for c in range(N_CORES):
        lo, hi = c * ROWS_PER_CORE, (c + 1) * ROWS_PER_CORE
        im = {
            f"x{m}": np.ascontiguousarray(
                inputs[f"outputs{m + 1}"][lo:hi], dtype=np.float32
            )
            for m in range(M)
        }
        im["colscale"] = colscale
        in_maps.append(im)
    res = run_bass_kernel_spmd(nc, in_maps, core_ids, **run_kwargs)
    qs = []
    for c in range(N_CORES):
        qc = np.asarray(res.results[c]["q_out"])  # [128, RT]
        qs.append(qc.T.reshape(-1))  # row = rt*128 + p order
    q_all = np.concatenate(qs).astype(np.float64)  # row = c*512 + rt*128 + p
    return q_all, res


def kernel(**inputs: np.ndarray) -> np.ndarray:
    q_all, _ = run_device_part(inputs)
    loss = SCALE * np.mean((q_all - float(M)) / 2.0)
    return np.float32(loss)
